# revision 7
# baseline (speedup 1.0000x reference)
"""MAB (multihead attention block) Trainium2 kernel, v5.

Sharding: 8 cores = 4 batches x 2 query-halves. Each core computes, for its
batch b and query half s (1024 queries), the full 8-head attention block:
    q = Q @ Wq.T + bq ; k = V @ Wk.T ; v = V @ Wv.T   (bv folded out; bk
    dropped: q.bk is constant across keys so softmax is invariant to it)
    S = q k^T / sqrt(512); masked softmax over keys; O = q + A @ v + bv
    out = O + relu(O @ Wo.T + bo)

v5 changes vs v4 (driven by HW microbenchmarks; the CoreSim cost model
mis-prices DoubleRow):
  - logits matmuls run in NORMAL mode: 64-row lhsT in the natural
    feature-major layout (feature f of head h sits at partition f%128 of
    chunk f//128 — identical to qt_sb/ot_sb), 128-col stationary => FWL
    fast weight load. DR only doubles contraction throughput, which the
    64-deep logits contraction cannot use, while paying a 2x LDWEIGHTS.
  - the fp8 q projection is gone: qt8 is an elementwise DVE cast of qt_sb
    (same layout). QT8/Wq8/bqp inputs disappear (-0.75 MB DMA).
  - k projection emits the natural layout directly (no host-side column
    permutation), DR over the 512-deep contraction (2 passes).
  - num / v / out projections keep DR (contraction 256/512: real 2x).
  - output is written bf16 (-1 MB DMA); host converts to f32.
  - mid-kernel softmax tails keep the DMA round-trip broadcast (measured
    faster than a PE ones-matmul broadcast); the final tail uses the PE
    path once the exp stream is done.

Precision plan (rel-err budget 2e-2): V/Wk/Wv and softmax weights (es)
fp8e4 (errors average over the 2048-key softmax); residual q path and
output projection bf16; PSUM accumulation f32; output bf16.

Schedule: one flat region paced by the ACT engine streaming one exp per
key-chunk step (~147us floor). Projection work is chopped into <=1us
passes interleaved into attention-step slack.
"""

import math
import os

import numpy as np

import concourse.bass as bass
import concourse.tile as tile
from concourse import bacc, mybir

F32 = mybir.dt.float32
BF16 = mybir.dt.bfloat16
FP8 = mybir.dt.float8e4
DR = mybir.MatmulPerfMode.DoubleRow

DIM = 512
NQ = 1024  # queries per core
NK = 2048  # keys per core
P = 128
FCH = DIM // P  # 4 feature chunks (= head pairs)
KD = DIM // P  # 4 contraction chunks (2 DoubleRow pairs)
TCH = NK // P  # 16 token/key chunks
QCH = NQ // 512  # 2 query chunks of 512
SCALE = 1.0 / math.sqrt(DIM)
MASK_NEG = -30000.0

# v_sb per-token-chunk column layout: 4 even-head blocks of 65 (v[64] | one),
# then 4 odd-head blocks of 128 (one | zeros[63] | v[64]), then 12 pad cols
# so the chunk stride is a multiple of 16 bytes (DoubleRow lhsT AP rule).
VW = 4 * 65 + 4 * 128 + 12  # 784
VUSED = 772
OUT_DTYPE = BF16
EVEN_OFF = [65 * i for i in range(4)]
ODD_OFF = [260 + 128 * i for i in range(4)]

INPUT_SPECS = {
    "QT": ((DIM, NQ), BF16),
    "VT": ((DIM, NK), FP8),
    "WqT": ((DIM, DIM), BF16),
    "Wk8": ((DIM, DIM), FP8),   # natural WkT, fp8
    "WvTp": ((DIM, VUSED), FP8),
    "WoT": ((DIM, DIM), BF16),
    "bq": ((DIM,), F32),
    "bv": ((DIM,), F32),
    "bo": ((DIM,), F32),
    "mlog": ((NK,), F32),
}


def emit(ctx, tc, io):
    """Emit the kernel. io: dict name -> DRAM AP (inputs + 'outT')."""
    nc = tc.nc
    AF = mybir.ActivationFunctionType
    OP = mybir.AluOpType

    consts = ctx.enter_context(tc.tile_pool(name="consts", bufs=1))
    bigs = ctx.enter_context(tc.tile_pool(name="bigs", bufs=1))

    # warm the ACT exp table early so the ~2.7us table load overlaps DMA
    warm = consts.tile([1, 1], F32)
    nc.vector.memset(warm, 0.0)
    nc.scalar.activation(warm, warm, AF.Exp)

    # all-ones stationary operand for the PE denominator-broadcast
    ones_sb = consts.tile([P, 64], BF16)
    nc.vector.memset(ones_sb, 1.0)

    # ---- weights / inputs ------------------------------------------------
    # Few, large DMAs on the SP HWDGE FIFO, strictly in first-use order.
    # (ACT-queue loads measured worse: in the steady state they queue
    # behind the previous body's entire exp stream.)
    vt_src = io["VT"].rearrange("(kd p) t -> p kd t", p=P)
    vtin = bigs.tile([P, KD, NK], FP8)
    nc.sync.dma_start(vtin[:, :, 0:512], vt_src[:, :, 0:512])
    wk8_sb = bigs.tile([P, KD, DIM], FP8)
    nc.sync.dma_start(wk8_sb, io["Wk8"].rearrange("(kd p) f -> p kd f", p=P))
    wq_sb = bigs.tile([P, KD, DIM], BF16)
    nc.sync.dma_start(wq_sb, io["WqT"].rearrange("(kd p) f -> p kd f", p=P))
    qtin = bigs.tile([P, KD, NQ], BF16)
    nc.sync.dma_start(qtin[:, :, 0:512], io["QT"].rearrange(
        "(kd p) t -> p kd t", p=P)[:, :, 0:512])
    bq_sb = consts.tile([P, FCH], F32)
    nc.sync.dma_start(bq_sb, io["bq"].rearrange("(c p) -> p c", p=P))
    mlog_sb = consts.tile([P, TCH], F32)
    nc.sync.dma_start(mlog_sb, io["mlog"].rearrange("(c p) -> p c", p=P))
    nc.sync.dma_start(vtin[:, :, 512:1024], vt_src[:, :, 512:1024])
    wvp_sb = bigs.tile([P, KD, VUSED], FP8)
    nc.sync.dma_start(wvp_sb, io["WvTp"].rearrange("(kd p) f -> p kd f", p=P))
    nc.sync.dma_start(vtin[:, :, 1024:1536], vt_src[:, :, 1024:1536])
    bv_sb = consts.tile([P, FCH], F32)
    nc.sync.dma_start(bv_sb, io["bv"].rearrange("(c p) -> p c", p=P))
    nc.sync.dma_start(vtin[:, :, 1536:2048], vt_src[:, :, 1536:2048])
    bo_sb = consts.tile([P, FCH], F32)
    nc.sync.dma_start(bo_sb, io["bo"].rearrange("(c p) -> p c", p=P))
    nc.sync.dma_start(qtin[:, :, 512:1024], io["QT"].rearrange(
        "(kd p) t -> p kd t", p=P)[:, :, 512:1024])
    wo_sb = bigs.tile([P, KD, DIM], BF16)
    nc.sync.dma_start(wo_sb, io["WoT"].rearrange("(kd p) f -> p kd f", p=P))

    # ---- persistent results ----------------------------------------------
    v_sb = bigs.tile([P, TCH, VW], FP8)
    kt8 = bigs.tile([P, FCH, NK], FP8)      # natural layout k (no bias)
    qt8 = bigs.tile([P, FCH, NQ], FP8)      # fp8 cast of qt_sb
    qt_sb = bigs.tile([P, FCH, NQ], BF16)   # residual-path q (feature-major)
    ot_sb = bigs.tile([P, FCH, NQ], BF16)

    # ---- pools -----------------------------------------------------------
    ps_s = ctx.enter_context(tc.tile_pool(name="ps_s", bufs=3, space="PSUM"))
    ps_n = ctx.enter_context(tc.tile_pool(name="ps_n", bufs=2, space="PSUM"))
    att = ctx.enter_context(tc.tile_pool(name="att", bufs=4))
    sm = ctx.enter_context(tc.tile_pool(name="sm", bufs=3))
    dr = ctx.enter_context(tc.tile_pool(name="dr", bufs=2, space="DRAM"))

    # ---- projection passes (transient users of the ps_s ring) ------------
    def v_pass(t):
        """Project v for key chunk t: token-major [128 tokens, 772]."""
        ps_v = ps_s.tile([P, VUSED], F32, tag="s", padded_shape=[P, 1024],
                         name="ps_v")
        for g in range(2):  # DoubleRow kd pairs
            lhsT = vtin[:, 2 * g:2 * g + 2, t * P:(t + 1) * P]
            nc.tensor.matmul(
                ps_v[:, 0:512], lhsT, wvp_sb[:, 2 * g:2 * g + 2, 0:512],
                start=(g == 0), stop=(g == 1), perf_mode=DR,
            )
            nc.tensor.matmul(
                ps_v[:, 512:VUSED], lhsT, wvp_sb[:, 2 * g:2 * g + 2, 512:VUSED],
                start=(g == 0), stop=(g == 1), perf_mode=DR,
            )
        nc.vector.tensor_copy(v_sb[:, t, 0:VUSED], ps_v)

    def ones_pair(c):
        """Set the denominator ones-columns for key chunks 2c, 2c+1."""
        ev = v_sb[:, 2 * c:2 * c + 2, 0:260].rearrange(
            "p t (e c) -> p t e c", c=65)[:, :, :, 64]
        nc.vector.memset(ev, 1.0)
        od = v_sb[:, 2 * c:2 * c + 2, 260:772].rearrange(
            "p t (o c) -> p t o c", c=128)[:, :, :, 0]
        nc.vector.memset(od, 1.0)

    def k8_half(pr, n):
        """Project k head-pair pr for key cols n*512.. (natural layout, DR)."""
        ps = ps_s.tile([P, 512], F32, tag="s", padded_shape=[P, 1024], name="ps_k8")
        for g in range(2):
            nc.tensor.matmul(
                ps, wk8_sb[:, 2 * g:2 * g + 2, pr * P:(pr + 1) * P],
                vtin[:, 2 * g:2 * g + 2, n * 512:(n + 1) * 512],
                start=(g == 0), stop=(g == 1), perf_mode=DR,
            )
        nc.vector.tensor_copy(kt8[:, pr, n * 512:(n + 1) * 512], ps)

    def qbf_quarter(fc, n):
        """Residual-path q (bf16, feature-major) + fp8 cast for logits."""
        ps = ps_s.tile([P, 512], F32, tag="s", padded_shape=[P, 1024], name="ps_q")
        for kd in range(KD):
            nc.tensor.matmul(
                ps, wq_sb[:, kd, fc * P:(fc + 1) * P],
                qtin[:, kd, n * 512:(n + 1) * 512],
                start=(kd == 0), stop=(kd == KD - 1),
            )
        qsl = slice(n * 512, (n + 1) * 512)
        nc.vector.tensor_scalar_add(qt_sb[:, fc, qsl], ps, bq_sb[:, fc:fc + 1])
        nc.vector.tensor_copy(qt8[:, fc, qsl], qt_sb[:, fc, qsl])

    out_dst = io["outT"].rearrange("(fc p) q -> p fc q", p=P)

    def out_finish(ups, qc, ofc, use_act=False):
        qsl = slice(qc * 512, (qc + 1) * 512)
        r1 = sm.tile([P, 512], BF16, tag="r1")
        if use_act:  # end of kernel: exp stream is done, ACT is free
            nc.scalar.activation(r1, ups, AF.Relu, bias=bo_sb[:, ofc:ofc + 1])
        else:
            nc.vector.tensor_scalar(
                r1, ups, bo_sb[:, ofc:ofc + 1], 0.0, op0=OP.add, op1=OP.max
            )
        fin = sm.tile([P, 512], BF16, tag="fin")
        nc.vector.tensor_tensor(fin, r1, ot_sb[:, ofc, qsl], op=OP.add)
        if use_act:
            # post-exp-stream: ACT HWDGE queue is idle; keeps the SP FIFO
            # clear for the next body's input prefetch in the repeat chain
            nc.scalar.dma_start(out_dst[:, ofc, qsl], fin)
        else:
            nc.sync.dma_start(out_dst[:, ofc, qsl], fin)

    def out_quarter(qc, ofc, use_act=False):
        """Full output-projection block for (qc, ofc) via the s ring."""
        qsl = slice(qc * 512, (qc + 1) * 512)
        ups = ps_s.tile([P, 512], F32, tag="s", padded_shape=[P, 1024], name="ups")
        for ifc in range(FCH):
            nc.tensor.matmul(
                ups, wo_sb[:, ifc, ofc * P:(ofc + 1) * P], ot_sb[:, ifc, qsl],
                start=(ifc == 0), stop=(ifc == FCH - 1),
            )
        out_finish(ups, qc, ofc, use_act=use_act)

    # ---- attention --------------------------------------------------------
    state = {}

    def att_begin(pr, qc):
        state["num0"] = ps_n.tile([P, 512], F32, tag="num", name="num0")
        state["num1"] = ps_n.tile([P, 512], F32, tag="num", name="num1")

    def att_step(pr, qc, kc):
        """Logits for both heads of pair pr, key chunk kc (normal mode)."""
        s_ps = ps_s.tile([P, 1024], F32, tag="s", name="s_ps")
        for hh in range(2):
            nc.tensor.matmul(
                s_ps[:, hh * 512:(hh + 1) * 512],
                kt8[64 * hh:64 * hh + 64, pr, kc * P:(kc + 1) * P],
                qt8[64 * hh:64 * hh + 64, pr, qc * 512:(qc + 1) * 512],
                start=True, stop=True, tile_position=(64 * hh, 0),
            )
        if kc % 2 == 0:
            state["es2"] = att.tile([P, 2, 1024], FP8, tag="es", name="es2")
        es2 = state["es2"]
        nc.scalar.activation(es2[:, kc % 2, :], s_ps, AF.Exp,
                             bias=mlog_sb[:, kc:kc + 1], scale=SCALE)

    def num_pair(pr, qc, c, num0, num1, es2):
        """Normal-mode numerator over the (2c, 2c+1) chunk pair (16-deep)."""
        for tt in (2 * c, 2 * c + 1):
            nc.tensor.matmul(
                num0[0:65, :], v_sb[:, tt, EVEN_OFF[pr]:EVEN_OFF[pr] + 65],
                es2[:, tt % 2, 0:512],
                start=(tt == 0), stop=(tt == TCH - 1),
            )
            nc.tensor.matmul(
                num1, v_sb[:, tt, ODD_OFF[pr]:ODD_OFF[pr] + 128],
                es2[:, tt % 2, 512:1024],
                start=(tt == 0), stop=(tt == TCH - 1),
            )

    def att_tail(pr, qc, fast=False):
        num0, num1 = state["num0"], state["num1"]
        qsl = slice(qc * 512, (qc + 1) * 512)
        if not fast:
            rec0 = sm.tile([65, 512], F32, tag="rec0")
            nc.vector.reciprocal(rec0[64:65, :], num0[64:65, :])
            rec1 = sm.tile([P, 512], F32, tag="rec1")
            nc.vector.reciprocal(rec1[0:1, :], num1[0:1, :])
        nab = sm.tile([P, 512], F32, tag="nab")
        nc.vector.tensor_copy(nab[0:64, :], num0[0:64, :])
        nc.vector.tensor_copy(nab[64:128, :], num1[64:128, :])
        if fast:
            # final tail: PE is idle, broadcast the reciprocal rows with a
            # K=1 bf16 ones-matmul (lower latency than the DMA round trip)
            rb0 = sm.tile([65, 512], BF16, tag="rb0")
            rb1 = sm.tile([P, 512], BF16, tag="rb1")
            with nc.allow_low_precision(reason="softmax 1/den scale, 2e-2 budget"):
                nc.vector.reciprocal(rb0[64:65, :], num0[64:65, :])
                nc.vector.reciprocal(rb1[0:1, :], num1[0:1, :])
            bca_ps = ps_s.tile([P, 512], F32, tag="s", padded_shape=[P, 1024],
                               name="bca_ps")
            nc.tensor.matmul(
                bca_ps[0:64, :], ones_sb[64:65, :], rb0[64:65, :],
                start=True, stop=True, tile_position=(64, 0),
            )
            nc.tensor.matmul(
                bca_ps[64:128, :], ones_sb[0:1, :], rb1[0:1, :],
                start=True, stop=True, tile_position=(0, 64),
            )
            bca = bca_ps
        else:
            # mid-kernel: DMA round-trip broadcast, fully off the PE path
            dr2 = dr.tile([2, 512], F32, tag="drec")
            nc.sync.dma_start(dr2[0:1, :], rec0[64:65, :])
            nc.sync.dma_start(dr2[1:2, :], rec1[0:1, :])
            bca = sm.tile([P, 512], F32, tag="bca")
            nc.sync.dma_start(bca[0:64, :], dr2[0:1, :].broadcast_to([64, 512]))
            nc.sync.dma_start(bca[64:128, :], dr2[1:2, :].broadcast_to([64, 512]))
        t1 = sm.tile([P, 512], BF16, tag="t1")
        nc.vector.tensor_tensor(t1, nab, bca, op=OP.mult)
        nc.vector.scalar_tensor_tensor(
            ot_sb[:, pr, qsl], t1, bv_sb[:, pr:pr + 1], qt_sb[:, pr, qsl],
            op0=OP.add, op1=OP.add,
        )

    # ---- fused schedule ---------------------------------------------------
    # prologue: what attention step 0 needs -- kt8 pr0 cols 0-1023 and
    # qt8 pr0 qc0. The v chunks stream inside block (0,0), two per step,
    # always one pair ahead of the numerator matmuls that consume them.
    k8_half(0, 0)
    k8_half(0, 1)
    qbf_quarter(0, 0)

    K8 = k8_half
    QB = qbf_quarter
    inserts = {
        # block (qc,pr) carries: its own cols-1024+ k8 spill, the next
        # pair's cols 0-1023 k8, and the next pair's qbf(+cast).
        (0, 0, 2): lambda: K8(0, 2),
        (0, 0, 4): lambda: K8(0, 3),
        (0, 0, 6): lambda: K8(1, 0),
        (0, 0, 8): lambda: K8(1, 1),
        (0, 0, 10): lambda: QB(1, 0),
        (0, 0, 12): lambda: QB(0, 1),
        (0, 1, 2): lambda: K8(1, 2),
        (0, 1, 4): lambda: K8(1, 3),
        (0, 1, 6): lambda: K8(2, 0),
        (0, 1, 8): lambda: K8(2, 1),
        (0, 1, 10): lambda: QB(2, 0),
        (0, 1, 12): lambda: QB(1, 1),
        (0, 2, 2): lambda: K8(2, 2),
        (0, 2, 4): lambda: K8(2, 3),
        (0, 2, 6): lambda: K8(3, 0),
        (0, 2, 8): lambda: K8(3, 1),
        (0, 2, 10): lambda: QB(3, 0),
        (0, 2, 12): lambda: QB(2, 1),
        (0, 3, 2): lambda: K8(3, 2),
        (0, 3, 4): lambda: K8(3, 3),
        (0, 3, 8): lambda: QB(3, 1),
        (1, 0, 3): lambda: out_quarter(0, 0),
        (1, 0, 7): lambda: out_quarter(0, 1),
        (1, 0, 11): lambda: out_quarter(0, 2),
        (1, 0, 15): lambda: out_quarter(0, 3),
    }

    deferred = None  # (pr, qc, num0, num1, es2) of the previous block's last pair
    for qc in range(QCH):
        for pr in range(FCH):
            att_begin(pr, qc)
            num0, num1 = state["num0"], state["num1"]
            for kc in range(TCH):
                att_step(pr, qc, kc)
                if kc == 0 and deferred is not None:
                    # previous block: last numerator pair, then its softmax
                    # tail -- after this block's first logits so the exp
                    # stream never waits on them
                    dpr, dqc, dn0, dn1, des2 = deferred
                    num_pair(dpr, dqc, TCH // 2 - 1, dn0, dn1, des2)
                    state["num0"], state["num1"] = dn0, dn1
                    att_tail(dpr, dqc)
                    state["num0"], state["num1"] = num0, num1
                    deferred = None
                if kc % 2 == 1:
                    if kc == TCH - 1:
                        deferred = (pr, qc, num0, num1, state["es2"])
                    else:
                        num_pair(pr, qc, kc // 2, num0, num1, state["es2"])
                if qc == 0 and pr == 0:
                    if kc == 0:
                        v_pass(0)
                        v_pass(1)
                        ones_pair(0)
                    elif kc < 15:
                        v_pass(kc + 1)
                        if kc % 2 == 0:
                            ones_pair(kc // 2)
                ins = inserts.get((qc, pr, kc))
                if ins is not None:
                    ins()
    dpr, dqc, dn0, dn1, des2 = deferred
    num_pair(dpr, dqc, TCH // 2 - 1, dn0, dn1, des2)
    state["num0"], state["num1"] = dn0, dn1
    att_tail(dpr, dqc, fast=True)
    for ofc in range(FCH):
        out_quarter(1, ofc, use_act=True)


def make_core_inputs(Q, V, mask, Wq, bq, Wk, bk, Wv, bv, Wo, bo, core):
    import ml_dtypes

    BF = ml_dtypes.bfloat16
    F8 = ml_dtypes.float8_e4m3fn
    b, s = divmod(core, 2)
    f32 = np.float32
    QT = np.ascontiguousarray(Q[b, s * NQ:(s + 1) * NQ, :].T)
    VT = np.ascontiguousarray(V[b].T).astype(F8)
    WvT = np.ascontiguousarray(Wv.T, dtype=f32)
    WvTp = np.zeros((DIM, VUSED), dtype=f32)
    for i in range(4):  # even heads 2i
        WvTp[:, EVEN_OFF[i]:EVEN_OFF[i] + 64] = WvT[:, (2 * i) * 64:(2 * i + 1) * 64]
    for i in range(4):  # odd heads 2i+1
        WvTp[:, ODD_OFF[i] + 64:ODD_OFF[i] + 128] = WvT[:, (2 * i + 1) * 64:(2 * i + 2) * 64]
    mlog = np.where(np.asarray(mask[b], bool), 0.0, MASK_NEG).astype(f32)
    return {
        "QT": QT.astype(BF),
        "VT": VT,
        "WqT": np.ascontiguousarray(Wq.T, dtype=f32).astype(BF),
        "Wk8": np.ascontiguousarray(Wk.T, dtype=f32).astype(F8),
        "WvTp": WvTp.astype(F8),
        "WoT": np.ascontiguousarray(Wo.T).astype(BF),
        "bq": np.asarray(bq, dtype=f32),
        "bv": np.asarray(bv, dtype=f32),
        "bo": np.asarray(bo, dtype=f32),
        "mlog": mlog,
    }


_CACHE = {}


def build_program():
    if "nc" in _CACHE:
        return _CACHE["nc"]
    from contextlib import ExitStack

    nc = bacc.Bacc("TRN2", target_bir_lowering=False, debug=False)
    io = {}
    for name, (shape, dt) in INPUT_SPECS.items():
        io[name] = nc.dram_tensor(name, list(shape), dt, kind="ExternalInput").ap()
    io["outT"] = nc.dram_tensor("outT", [DIM, NQ], BF16, kind="ExternalOutput").ap()
    with tile.TileContext(nc) as tc:
        with ExitStack() as ctx:
            emit(ctx, tc, io)
    nc.compile()
    _CACHE["nc"] = nc
    return nc


def kernel(Q, V, mask, Wq, bq, Wk, bk, Wv, bv, Wo, bo):
    from concourse.bass_utils import run_bass_kernel_spmd

    nc = build_program()
    args = (Q, V, mask, Wq, bq, Wk, bk, Wv, bv, Wo, bo)
    in_maps = [make_core_inputs(*args, core=c) for c in range(8)]
    res = run_bass_kernel_spmd(
        nc, in_maps, core_ids=list(range(8)),
        trace=bool(int(os.environ.get("KTRACE", "0"))),
    )
    _CACHE["last_result"] = res
    B = 4
    out = np.empty((B, 2 * NQ, DIM), np.float32)
    for c in range(8):
        b, s = divmod(c, 2)
        out[b, s * NQ:(s + 1) * NQ, :] = np.asarray(
            res.results[c]["outT"], dtype=np.float32).T
    return out
